# revision 1
# baseline (speedup 1.0000x reference)
"""Trainium2 Bass kernel for CrossModalAttention (attention + residual + LayerNorm).

Math: the reference concatenates [query, key_value], projects Q/K/V, attends with an
additive -10000 mask on key columns < N_q, and keeps only the query-row outputs.
With scores ~ N(0,1), exp(score - 10000 - rowmax) underflows to exactly 0 in fp32,
so the computation is exactly standard cross-attention:
    Q = query @ Wq + bq ; K = key_value @ Wk + bk ; V = key_value @ Wv + bv
    C = softmax(Q K^T / 8) V ;  out = LayerNorm(query + C) * gamma + beta

Sharding: 8 cores = 2 batches x 4 query-blocks of 512 rows. Each core computes the
K/V projections for its batch (duplicated across the 4 cores of a batch) and full
8-head attention + LN for its query block.

Per-core kernel layout (S^T flash attention):
  - QT[dout, q] / KT[dout, keys] computed directly transposed (W as stationary).
  - scores S^T[k, q] per head via K=64 matmuls, two heads packed in the PE array
    with tile_position row tiling.
  - exp on ScalarE straight out of PSUM (scale=1/8 folded in), fp16 out.
  - PV with ones-augmented V (M=65): row 64 accumulates the softmax denominator.
  - PE-transpose C^T -> C, normalize by 1/l, residual + LayerNorm in fp32.
"""

import os
import sys

import numpy as np

try:
    import concourse.bass as bass  # noqa: F401
except ImportError:
    for _p in ("/opt/trn_rl_repo", "/root/.axon_site/_ro/trn_rl_repo"):
        if os.path.isdir(_p):
            sys.path.insert(0, _p)
            break
    import concourse.bass as bass  # noqa: F401

import concourse.tile as tile
from concourse import bacc, bass_utils, mybir
from concourse.masks import make_identity

F32 = mybir.dt.float32
F16 = mybir.dt.float16
AF = mybir.ActivationFunctionType
ALU = mybir.AluOpType

B, N_Q, N_KV, D, H, HD = 2, 2048, 2048, 512, 8, 64
N_CORES = 8
QBLK = N_Q // 4          # 512 query rows per core
DC = D // 128            # 4 partition chunks of the model dim
KC = N_KV // 128         # 16 key chunks
LN_EPS = 1e-5
SM_SCALE = 1.0 / np.sqrt(HD)


def _build_program(trace=False):
    nc = bacc.Bacc("TRN2", target_bir_lowering=False, debug=False,
                   num_devices=N_CORES)

    def din(name, shape, dt=F16):
        return nc.dram_tensor(name, shape, dt, kind="ExternalInput").ap()

    xqT = din("xqT", [D, QBLK])           # query block, transposed
    xq = din("xq", [QBLK, D], F32)        # query block (residual)
    kvT = din("kvT", [D, N_KV])           # key_value, transposed
    wq = din("wq", [D, D])
    wk = din("wk", [D, D])
    wv = din("wv", [D, D])
    bqc = din("bqc", [128, DC], F32)      # bq chunked [128, 4]
    bkc = din("bkc", [128, DC], F32)
    bvb = din("bvb", [128, D], F32)       # bv broadcast to 128 partitions
    gammab = din("gammab", [128, D], F32)
    betab = din("betab", [128, D], F32)
    out = nc.dram_tensor("out", [QBLK, D], F32, kind="ExternalOutput").ap()

    with tile.TileContext(nc) as tc:
        with (
            tc.tile_pool(name="persist", bufs=1) as pp,
            tc.tile_pool(name="work", bufs=2) as wkp,
            tc.tile_pool(name="small", bufs=8) as smp,
            tc.tile_pool(name="scratch_ps", bufs=2, space="PSUM") as sps,
            tc.tile_pool(name="sc_ps", bufs=2, space="PSUM") as scps,
            tc.tile_pool(name="pv_ps", bufs=1, space="PSUM") as pvps,
        ):
            # ---- loads: ordered so QT/KT projections can start ASAP ----
            w_sb = {}
            for nm in ("wq", "wk", "wv"):
                w_sb[nm] = [pp.tile([128, D], F16, name=f"{nm}{c}", tag=f"{nm}{c}")
                            for c in range(DC)]
            xqt_sb = [pp.tile([128, QBLK], F16, name=f"xqt{c}", tag=f"xqt{c}") for c in range(DC)]
            kvt_sb = [pp.tile([128, N_KV], F16, name=f"kvt{c}", tag=f"kvt{c}") for c in range(DC)]
            xq_sb = [pp.tile([128, D], F32, name=f"xq{q}", tag=f"xq{q}") for q in range(4)]
            bqc_sb = pp.tile([128, DC], F32, name="bqc", tag="bqc")
            bkc_sb = pp.tile([128, DC], F32, name="bkc", tag="bkc")
            bvb_sb = pp.tile([128, D], F32, name="bvb", tag="bvb")
            gam_sb = pp.tile([128, D], F32, name="gam", tag="gam")
            bet_sb = pp.tile([128, D], F32, name="bet", tag="bet")
            # sync queue: what QT/KT projections need first
            nc.sync.dma_start(bqc_sb[:], bqc)
            nc.sync.dma_start(bkc_sb[:], bkc)
            for c in range(DC):
                nc.sync.dma_start(xqt_sb[c][:], xqT[c * 128:(c + 1) * 128, :])
                nc.sync.dma_start(w_sb["wq"][c][:], wq[c * 128:(c + 1) * 128, :])
            for c in range(DC):
                nc.sync.dma_start(w_sb["wk"][c][:], wk[c * 128:(c + 1) * 128, :])
            for c in range(DC):
                nc.sync.dma_start(kvt_sb[c][:, 0:N_KV // 2],
                                  kvT[c * 128:(c + 1) * 128, 0:N_KV // 2])
            # gpsimd queue: second kvT half + remaining weights, then LN consts
            for c in range(DC):
                nc.gpsimd.dma_start(kvt_sb[c][:, N_KV // 2:],
                                    kvT[c * 128:(c + 1) * 128, N_KV // 2:])
            nc.gpsimd.dma_start(bvb_sb[:], bvb)
            for c in range(DC):
                nc.gpsimd.dma_start(w_sb["wv"][c][:], wv[c * 128:(c + 1) * 128, :])
            ident = pp.tile([128, 128], F32, name="ident", tag="ident")
            make_identity(nc, ident[:])
            for q in range(4):
                nc.gpsimd.dma_start(xq_sb[q][:], xq[q * 128:(q + 1) * 128, :])
            nc.gpsimd.dma_start(gam_sb[:], gammab)
            nc.gpsimd.dma_start(bet_sb[:], betab)
            eps_sb = pp.tile([128, 1], F32, name="eps", tag="eps")
            nc.vector.memset(eps_sb[:], float(LN_EPS))

            qt_sb = [pp.tile([128, QBLK], F16, name=f"qt{m}", tag=f"qt{m}") for m in range(DC)]
            kt_sb = [pp.tile([128, N_KV], F16, name=f"kt{m}", tag=f"kt{m}") for m in range(DC)]
            vaug_sb = [pp.tile([128, H * (HD + 1)], F16, name=f"va{t}", tag=f"va{t}")
                       for t in range(KC)]
            c_sb = [pp.tile([128, D], F32, name=f"csb{q}", tag=f"csb{q}") for q in range(4)]

            def proj_qt(m):
                ps = sps.tile([128, QBLK], F32, name="ps_qt", tag="scratch")
                for c in range(DC):
                    nc.tensor.matmul(
                        ps[:], w_sb["wq"][c][:, m * 128:(m + 1) * 128],
                        xqt_sb[c][:], start=(c == 0), stop=(c == DC - 1))
                nc.vector.tensor_scalar(
                    out=qt_sb[m][:], in0=ps[:], scalar1=bqc_sb[:, m:m + 1],
                    scalar2=None, op0=ALU.add)

            def proj_kt(m):
                for n in range(N_KV // 512):
                    ps = sps.tile([128, 512], F32, name="ps_kt", tag="scratch")
                    for c in range(DC):
                        nc.tensor.matmul(
                            ps[:], w_sb["wk"][c][:, m * 128:(m + 1) * 128],
                            kvt_sb[c][:, n * 512:(n + 1) * 512],
                            start=(c == 0), stop=(c == DC - 1))
                    nc.vector.tensor_scalar(
                        out=kt_sb[m][:, n * 512:(n + 1) * 512], in0=ps[:],
                        scalar1=bkc_sb[:, m:m + 1], scalar2=None, op0=ALU.add)

            def proj_v(t):
                ps = sps.tile([128, D], F32, name="ps_v", tag="scratch")
                for c in range(DC):
                    nc.tensor.matmul(
                        ps[:], kvt_sb[c][:, t * 128:(t + 1) * 128],
                        w_sb["wv"][c][:], start=(c == 0), stop=(c == DC - 1))
                va3 = vaug_sb[t][:].rearrange("p (h d) -> p h d", h=H)
                nc.vector.tensor_tensor(
                    out=va3[:, :, 0:HD],
                    in0=ps[:].rearrange("p (h d) -> p h d", h=H),
                    in1=bvb_sb[:].rearrange("p (h d) -> p h d", h=H),
                    op=ALU.add)
                nc.vector.memset(vaug_sb[t][:, HD::HD + 1], 1.0)

            def scores_exp(g, kc):
                psc = scps.tile([128, 2 * QBLK], F32, name="psc", tag="sc")
                for j in range(2):
                    nc.tensor.matmul(
                        psc[:, j * QBLK:(j + 1) * QBLK],
                        kt_sb[g][j * 64:(j + 1) * 64, kc * 128:(kc + 1) * 128],
                        qt_sb[g][j * 64:(j + 1) * 64, :],
                        start=True, stop=True, tile_position=(j * 64, 0))
                pt = wkp.tile([128, 2 * QBLK], F16, name="pt", tag="pt")
                nc.scalar.activation(pt[:], psc[:], AF.Exp, scale=float(SM_SCALE))
                return pt

            def pv(g, kc, ppv, pt):
                for j in range(2):
                    h = 2 * g + j
                    nc.tensor.matmul(
                        ppv[j][:],
                        vaug_sb[kc][:, h * (HD + 1):(h + 1) * (HD + 1)],
                        pt[:, j * QBLK:(j + 1) * QBLK],
                        start=(kc == 0), stop=(kc == KC - 1))

            def finish_pair(g, ppv, then_ln=False):
                cts = []
                for j in range(2):
                    ct = wkp.tile([HD + 1, QBLK], F32, name="ct", tag="ct")
                    nc.vector.tensor_copy(ct[:], ppv[j][:])
                    cts.append(ct)
                for q in range(4):
                    for j in range(2):
                        h = 2 * g + j
                        ptr = sps.tile([128, HD + 1], F32, name="ptr", tag="scratch")
                        nc.tensor.transpose(
                            ptr[:], cts[j][:, q * 128:(q + 1) * 128],
                            ident[0:HD + 1, 0:HD + 1])
                        linv = smp.tile([128, 1], F32, name="linv", tag="linv")
                        nc.vector.reciprocal(linv[:], ptr[:, HD:HD + 1])
                        nc.vector.tensor_scalar(
                            out=c_sb[q][:, h * HD:(h + 1) * HD],
                            in0=ptr[:, 0:HD], scalar1=linv[:], scalar2=None,
                            op0=ALU.mult)
                    if then_ln:
                        layer_norm(q)

            # ---- residual + LayerNorm ----
            def layer_norm(q):
                    resid = wkp.tile([128, D], F32, name="resid", tag="resid")
                    rowsum = smp.tile([128, 1], F32, name="rowsum", tag="rowsum")
                    nc.vector.scalar_tensor_tensor(
                        out=resid[:], in0=c_sb[q][:], scalar=0.0, in1=xq_sb[q][:],
                        op0=ALU.bypass, op1=ALU.add, accum_out=rowsum[:])
                    sq = wkp.tile([128, D], F32, name="sq", tag="sq")
                    sqs = smp.tile([128, 1], F32, name="sqs", tag="sqs")
                    nc.scalar.activation(sq[:], resid[:], AF.Square, accum_out=sqs[:])
                    mu = smp.tile([128, 1], F32, name="mu", tag="mu")
                    nc.vector.tensor_scalar_mul(mu[:], rowsum[:], 1.0 / D)
                    musq = smp.tile([128, 1], F32, name="musq", tag="musq")
                    nc.vector.tensor_tensor(out=musq[:], in0=mu[:], in1=mu[:], op=ALU.mult)
                    var = smp.tile([128, 1], F32, name="var", tag="var")
                    nc.vector.scalar_tensor_tensor(
                        out=var[:], in0=sqs[:], scalar=1.0 / D, in1=musq[:],
                        op0=ALU.mult, op1=ALU.subtract)
                    std = smp.tile([128, 1], F32, name="std", tag="std")
                    nc.scalar.activation(std[:], var[:], AF.Sqrt, bias=eps_sb[:])
                    inv = smp.tile([128, 1], F32, name="inv", tag="inv")
                    nc.vector.reciprocal(inv[:], std[:])
                    xcn = wkp.tile([128, D], F32, name="xcn", tag="xcn")
                    nc.vector.tensor_scalar(
                        out=xcn[:], in0=resid[:], scalar1=mu[:], scalar2=inv[:],
                        op0=ALU.subtract, op1=ALU.mult)
                    t2 = wkp.tile([128, D], F32, name="t2", tag="t2")
                    nc.vector.tensor_tensor(out=t2[:], in0=xcn[:], in1=gam_sb[:], op=ALU.mult)
                    o = wkp.tile([128, D], F32, name="o", tag="o")
                    nc.vector.tensor_tensor(out=o[:], in0=t2[:], in1=bet_sb[:], op=ALU.add)
                    nc.sync.dma_start(out[q * 128:(q + 1) * 128, :], o[:])


            # ---- phase schedule ----
            # pair 0 with V projection interleaved; later pairs follow their
            # own QT/KT projection; transposes of pair g overlap pair g+1.
            proj_qt(0)
            proj_kt(0)
            pending = None  # (g, ppv) awaiting finish_pair
            for g in range(DC):
                ppv = [pvps.tile([HD + 1, QBLK], F32, name=f"pv{j}", tag=f"pv{j}")
                       for j in range(2)]
                pt_prev = None
                for kc in range(KC):
                    if g == 0:
                        proj_v(kc)
                    pt = scores_exp(g, kc)
                    if pt_prev is not None:
                        pv(g, kc - 1, ppv, pt_prev)
                    pt_prev = pt
                pv(g, KC - 1, ppv, pt_prev)
                if g + 1 < DC:
                    proj_qt(g + 1)
                    proj_kt(g + 1)
                if pending is not None:
                    finish_pair(*pending)
                pending = (g, ppv)
            finish_pair(*pending, then_ln=True)

    nc.compile()
    return nc


_PROGRAM = None


def _get_program():
    global _PROGRAM
    if _PROGRAM is None:
        _PROGRAM = _build_program()
    return _PROGRAM


def _make_in_maps(query, key_value, Wq, bq, Wk, bk, Wv, bv, ln_gamma, ln_beta):
    f16 = np.float16
    f32 = np.float32
    wq16 = np.ascontiguousarray(Wq, dtype=f16)
    wk16 = np.ascontiguousarray(Wk, dtype=f16)
    wv16 = np.ascontiguousarray(Wv, dtype=f16)
    bqc = np.ascontiguousarray(bq.reshape(DC, 128).T, dtype=f32)
    bkc = np.ascontiguousarray(bk.reshape(DC, 128).T, dtype=f32)
    bvb = np.ascontiguousarray(np.broadcast_to(bv[None, :], (128, D)), dtype=f32)
    gammab = np.ascontiguousarray(np.broadcast_to(ln_gamma[None, :], (128, D)), dtype=f32)
    betab = np.ascontiguousarray(np.broadcast_to(ln_beta[None, :], (128, D)), dtype=f32)
    kvT = [np.ascontiguousarray(key_value[b].T, dtype=f16) for b in range(B)]
    in_maps = []
    for core in range(N_CORES):
        b, qb = divmod(core, 4)
        blk = query[b, qb * QBLK:(qb + 1) * QBLK, :]
        in_maps.append({
            "xqT": np.ascontiguousarray(blk.T, dtype=f16),
            "xq": np.ascontiguousarray(blk, dtype=f32),
            "kvT": kvT[b],
            "wq": wq16, "wk": wk16, "wv": wv16,
            "bqc": bqc, "bkc": bkc, "bvb": bvb,
            "gammab": gammab, "betab": betab,
        })
    return in_maps


def kernel(query, key_value, Wq, bq, Wk, bk, Wv, bv, ln_gamma, ln_beta,
           _trace=False, _trace_kwargs=None):
    args = [np.asarray(a, dtype=np.float32) for a in
            (query, key_value, Wq, bq, Wk, bk, Wv, bv, ln_gamma, ln_beta)]
    nc = _get_program()
    in_maps = _make_in_maps(*args)
    res = bass_utils.run_bass_kernel_spmd(
        nc, in_maps, core_ids=list(range(N_CORES)), trace=_trace,
        **(_trace_kwargs or {}))
    out = np.empty((B, N_Q, D), np.float32)
    for core in range(N_CORES):
        b, qb = divmod(core, 4)
        out[b, qb * QBLK:(qb + 1) * QBLK, :] = res.results[core]["out"]
    if _trace:
        return out, res
    return out

